# revision 19
# baseline (speedup 1.0000x reference)
"""Expert-parallel MoE kernel for Trainium2 (8 NeuronCores).

Strategy (matches the module's intent):
  - Host computes the (tiny) gating: logits -> softmax -> top-2 -> renormalized
    combine weights. This is the router / all-to-all dispatch plumbing.
  - The 8 experts are split along DFF into 16 half-experts and the halves
    are pair-scheduled for load balance: expert pairs (heaviest with
    lightest by routed-token count) share two cores, core 2i getting the
    front DFF-half of both experts of pair i and core 2i+1 the back half.
    Each half computes an exact partial of its expert's MLP output
    (MLP1+gelu act elementwise in f; MLP2 is a sum over f), already scaled
    by the combine weights; the host adds the two halves and scatter-adds
    into the output.  This balances per-core tokens to ~max(heavy)+
    max(light) instead of 2*max(all) at identical weight traffic.
  - Both matmuls run in bf16 with fp32 PSUM accumulation.

Layout: activations are kept feature-major on device (features on SBUF
partitions, tokens on the free dim) so both weight matrices are used in
their native layout as the stationary matmul operand and no transposes
are needed anywhere on device.

DMA schedule: everything the matmul stream consumes rides the sync-ring
HWDGE queue in consumption order (x_A slice 0, w1_A chunks smallest-
first, x_A slice 1, w2_A, combine weights A, then the unit-B tensors)
so ring FIFO implements priority; w1_A chunk sizes and the asymmetric
token split are tuned so the ramping DMA supply always stays ahead of
the PE's consumption cadence.  Measured on hardware: the matmul stream
runs gapless at the N/2.4GHz+NX floor; the remaining time is the
framework preamble (~7us), the DMA ramp to the first w1 chunk (~4us),
and the final store + epilogue (~4.5us).
"""

import os
import sys

sys.path.insert(0, "/opt/trn_rl_repo")

import numpy as np
import ml_dtypes

H = 768
E = 8
DFF = 3072
DFFH = DFF // 2   # per-unit (half-expert) ffn width
P = 128
HO = H // P       # 6 h-tiles
FOH = DFFH // P   # 12 f-tiles per unit
N_CORES = 8
N_WARMUP_MM = 44  # dummy matmuls to open the HAM clock gate during DMA ramp
# unit-A w1 arrives in f-blocks; small leading blocks match the ramping
# DMA supply rate to the matmul consumption cadence so the stream never
# stalls on a chunk arrival.  Unit B's weights arrive long before use and
# need no chunking.
FBLKS = [128] * 8 + [256] * 2
FBLK_STARTS = [0]
for _c in FBLKS:
    FBLK_STARTS.append(FBLK_STARTS[-1] + _c)
assert FBLK_STARTS[-1] == DFFH
NFBLK = len(FBLKS)
# j (128-col f-tile) -> (chunk index, col offset inside chunk)
J2FB = []
for _j in range(FOH):
    _c0 = _j * P
    for _fb in range(NFBLK):
        if FBLK_STARTS[_fb] <= _c0 < FBLK_STARTS[_fb + 1]:
            J2FB.append((_fb, _c0 - FBLK_STARTS[_fb]))
            break

LAST_RESULTS = None  # BassKernelResults of the most recent run (for test.py)
TRACE = False        # set True (e.g. by test.py) to profile the run


def _token_slices(C):
    """Split C tokens into PSUM-sized (<=512) slices.

    The split is asymmetric on purpose: the leading slice is ~65% so its
    matmul groups consume w1 chunks SLOWER than the ramping DMA supply
    delivers them (robustness against run-to-run DMA-ramp variance), and
    the final slice is small so the last output tile's store (on the
    kernel's critical tail) is cheap.
    """
    if C <= 256:
        return (C,)
    n_t = max(2, -(-C // 512))
    sizes = []
    left = C
    for k in range(n_t, 0, -1):
        if k == 1:
            s = left
        else:
            s = min(512, -(-int(left * 0.65) // 8) * 8)
        sizes.append(s)
        left -= s
    assert all(0 < s <= 512 for s in sizes) and sum(sizes) == C
    return tuple(sizes)


def _build(CA, TSA, CB, TSB, b2_zero=False):
    import concourse.mybir as mybir
    import concourse.tile as tile
    from concourse import bacc

    f32 = mybir.dt.float32
    bf16 = mybir.dt.bfloat16
    GELU = mybir.ActivationFunctionType.Gelu
    IDENT = mybir.ActivationFunctionType.Identity

    nc = bacc.Bacc("TRN2", target_bir_lowering=False, debug=False)

    # bf16 partial outputs halve the store traffic (host accumulates in
    # f32; the quantization adds ~0.2% rel err, well under the gate)
    out_dt = bf16 if b2_zero else f32

    units = [
        ("A", CA, TSA),
        ("B", CB, TSB),
    ]
    dram = {}
    for un, C, TS in units:
        NT = len(TS)
        dram[f"x{un}"] = nc.dram_tensor(
            f"x{un}", [NT, P, HO, max(TS)], bf16, kind="ExternalInput"
        ).ap()
        if un == "A":
            dram["w1A"] = [
                nc.dram_tensor(
                    f"w1Ac{fb}", [P, HO, FBLKS[fb]], bf16, kind="ExternalInput"
                ).ap()
                for fb in range(NFBLK)
            ]
        else:
            dram["w1B"] = nc.dram_tensor(
                "w1B", [P, HO, DFFH], bf16, kind="ExternalInput"
            ).ap()
        dram[f"w2{un}"] = nc.dram_tensor(
            f"w2{un}", [P, FOH, H], bf16, kind="ExternalInput"
        ).ap()
        dram[f"b1{un}"] = nc.dram_tensor(
            f"b1{un}", [P, FOH], f32, kind="ExternalInput"
        ).ap()
        dram[f"b2{un}"] = nc.dram_tensor(
            f"b2{un}", [P, HO], f32, kind="ExternalInput"
        ).ap()
        dram[f"wb{un}"] = nc.dram_tensor(
            f"wb{un}", [P, C], f32, kind="ExternalInput"
        ).ap()
        dram[f"out{un}"] = nc.dram_tensor(
            f"out{un}", [H, C], out_dt, kind="ExternalOutput"
        ).ap()

    with tile.TileContext(nc) as tc:
        with (
            tc.tile_pool(name="const", bufs=1) as const,
            tc.tile_pool(name="hmidp", bufs=1) as hmidp,
            tc.tile_pool(name="psum", bufs=7, space="PSUM") as psum,
            tc.tile_pool(name="wupp", bufs=1, space="PSUM") as wupp,
            tc.tile_pool(name="outp", bufs=4) as outp,
        ):
            # ---- PE warm-up: dummy matmuls so the HAM clock-gate opens while
            # the weight DMAs are still in flight.  The memset runs on gpsimd
            # (it leaves the framework preamble ~1us earlier than vector).
            scr = const.tile([P, P], bf16, name="scr", tag="scr")
            nc.gpsimd.memset(scr, 0.0)
            psd = wupp.tile([P, P], f32, name="psd", tag="psd")
            for _ in range(N_WARMUP_MM):
                nc.tensor.matmul(psd, lhsT=scr, rhs=scr, start=True, stop=True)

            # ---- DMA schedule.  Everything the compute stream consumes goes
            # on the sync ring (queue 1) in consumption order.  Ring FIFO =
            # priority.  The ACT ring (queue 10) only gets the tiny biases:
            # it has a 2-4.5us startup latency and only ~130 GB/s.  The
            # combine weights are pre-broadcast on the host to [P, C] (the
            # DRE replication broadcast ran at ~79 GB/s on the SWDGE queue
            # and stole SDMA time exactly while the first w1 chunks were in
            # flight).
            sb = {}
            for un, C, TS in units:
                sb[f"b1{un}"] = const.tile(
                    [P, FOH], f32, name=f"b1{un}", tag=f"b1{un}"
                )
                nc.scalar.dma_start(out=sb[f"b1{un}"], in_=dram[f"b1{un}"])
                if not b2_zero:
                    sb[f"b2{un}"] = const.tile(
                        [P, HO], f32, name=f"b2{un}", tag=f"b2{un}"
                    )
                    nc.scalar.dma_start(out=sb[f"b2{un}"], in_=dram[f"b2{un}"])

            for un, C, TS in units:
                xts = []
                for ti, tn in enumerate(TS):
                    t = const.tile(
                        [P, HO, tn], bf16, name=f"x{un}{ti}", tag=f"x{un}{ti}"
                    )
                    xts.append(t)
                sb[f"x{un}"] = xts
                # unit A: x slice 0, w1 chunks, x slice 1, w2, wb
                # unit B: everything arrives long before use; plain order
                nc.sync.dma_start(out=xts[0], in_=dram[f"x{un}"][0, :, :, : TS[0]])
                if un == "A":
                    w1cs = []
                    for fb in range(NFBLK):
                        t = const.tile(
                            [P, HO, FBLKS[fb]], bf16,
                            name=f"w1A{fb}", tag=f"w1A{fb}",
                        )
                        nc.sync.dma_start(out=t, in_=dram["w1A"][fb])
                        w1cs.append(t)
                    sb["w1A"] = w1cs
                else:
                    t = const.tile([P, HO, DFFH], bf16, name="w1B", tag="w1B")
                    nc.sync.dma_start(out=t, in_=dram["w1B"])
                    sb["w1B"] = t
                for ti, tn in list(enumerate(TS))[1:]:
                    nc.sync.dma_start(
                        out=xts[ti], in_=dram[f"x{un}"][ti, :, :, :tn]
                    )
                t = const.tile([P, FOH, H], bf16, name=f"w2{un}", tag=f"w2{un}")
                nc.sync.dma_start(out=t, in_=dram[f"w2{un}"])
                sb[f"w2{un}"] = t
                t = const.tile([P, C], f32, name=f"wb{un}", tag=f"wb{un}")
                nc.sync.dma_start(out=t, in_=dram[f"wb{un}"])
                sb[f"wb{un}"] = t

            # ---- compute: per unit, MLP1 then MLP2 --------------------------
            for un, C, TS in units:
                hmid = [
                    hmidp.tile([P, C], bf16, name=f"hm{un}{fo}", tag=f"hm{un}{fo}")
                    for fo in range(FOH)
                ]
                starts = np.cumsum([0] + list(TS))
                # MLP1: hmid[f, t] = gelu(sum_h W1[h,f] x[h,t] + b1[f])
                for ti, tn in enumerate(TS):
                    t0 = int(starts[ti])
                    for j in range(FOH):
                        ps = psum.tile([P, 512], f32, name="ps1", tag="ps")
                        for ho in range(HO):
                            if un == "A":
                                fb, joff = J2FB[j]
                                lhsT = sb["w1A"][fb][:, ho, joff : joff + P]
                            else:
                                lhsT = sb["w1B"][:, ho, j * P : (j + 1) * P]
                            nc.tensor.matmul(
                                ps[:, :tn],
                                lhsT=lhsT,
                                rhs=sb[f"x{un}"][ti][:, ho, :tn],
                                start=(ho == 0),
                                stop=(ho == HO - 1),
                            )
                        nc.scalar.activation(
                            hmid[j][:, t0 : t0 + tn],
                            ps[:, :tn],
                            GELU,
                            bias=sb[f"b1{un}"][:, j : j + 1],
                        )
                # MLP2 + combine scale
                for ti, tn in enumerate(TS):
                    t0 = int(starts[ti])
                    for i in range(HO):
                        ps = psum.tile([P, 512], f32, name="ps2", tag="ps")
                        for fo in range(FOH):
                            nc.tensor.matmul(
                                ps[:, :tn],
                                lhsT=sb[f"w2{un}"][:, fo, i * P : (i + 1) * P],
                                rhs=hmid[fo][:, t0 : t0 + tn],
                                start=(fo == 0),
                                stop=(fo == FOH - 1),
                            )
                        ot = outp.tile(
                            [P, 512], bf16 if b2_zero else f32, name="ot", tag="ot"
                        )
                        if b2_zero:
                            nc.vector.tensor_mul(
                                ot[:, :tn], ps[:, :tn],
                                sb[f"wb{un}"][:, t0 : t0 + tn],
                            )
                        else:
                            nc.scalar.activation(
                                ot[:, :tn], ps[:, :tn], IDENT,
                                bias=sb[f"b2{un}"][:, i : i + 1],
                            )
                            nc.vector.tensor_mul(
                                ot[:, :tn], ot[:, :tn],
                                sb[f"wb{un}"][:, t0 : t0 + tn],
                            )
                        nc.sync.dma_start(
                            out=dram[f"out{un}"][i * P : (i + 1) * P, t0 : t0 + tn],
                            in_=ot[:, :tn],
                        )

    nc.compile()
    return nc


def _pad_c(n):
    return max(-(-n // 16) * 16, 128)


def kernel(x, Wg, bg, W1, b1, W2, b2, top_k):
    global LAST_RESULTS
    from concourse import bass_utils

    x = np.asarray(x, dtype=np.float32)
    Wg = np.asarray(Wg, dtype=np.float32)
    bg = np.asarray(bg, dtype=np.float32)
    W1 = np.asarray(W1, dtype=np.float32)
    b1 = np.asarray(b1, dtype=np.float32)
    W2 = np.asarray(W2, dtype=np.float32)
    b2 = np.asarray(b2, dtype=np.float32)
    k = int(np.asarray(top_k))
    assert k == 2, f"kernel specialized for top_k=2, got {k}"
    assert E == N_CORES == 8

    b, s, h = x.shape
    T = b * s
    xf = x.reshape(T, h)

    # ---- host router (the all-to-all dispatch) ------------------------------
    logits = xf @ Wg + bg
    m = logits.max(axis=-1, keepdims=True)
    p = np.exp(logits - m)
    p /= p.sum(axis=-1, keepdims=True)
    i1 = np.argmax(p, axis=-1)
    p_masked = p.copy()
    p_masked[np.arange(T), i1] = -np.inf
    i2 = np.argmax(p_masked, axis=-1)
    denom = p[np.arange(T), i1] + p[np.arange(T), i2]

    tok_idx, tok_w = [], []
    for e in range(E):
        sel = np.where((i1 == e) | (i2 == e))[0]
        tok_idx.append(sel.astype(np.int64))
        tok_w.append((p[sel, e] / denom[sel]).astype(np.float32))
    cnts = np.array([len(t) for t in tok_idx])

    # pair heaviest with lightest: pair i -> cores 2i (front DFF half of
    # both experts) and 2i+1 (back half)
    order = np.argsort(-cnts, kind="stable")
    heavy = [int(order[i]) for i in range(4)]
    light = [int(order[7 - i]) for i in range(4)]
    CA = _pad_c(max(cnts[e] for e in heavy))
    CB = _pad_c(max(cnts[e] for e in light))
    TSA = _token_slices(CA)
    TSB = _token_slices(CB)

    b2_zero = not np.any(b2)

    # ---- per-core inputs ----------------------------------------------------
    bf = ml_dtypes.bfloat16

    def _x_tiles(e, C, TS):
        cnt = len(tok_idx[e])
        NT = len(TS)
        tstarts = np.concatenate([[0], np.cumsum(TS)]).astype(int)
        xfull = np.zeros((P, HO, C), dtype=bf)
        xfull[:, :, :cnt] = (
            np.ascontiguousarray(xf[tok_idx[e]].T).astype(bf)
            .reshape(HO, P, cnt)
            .transpose(1, 0, 2)
        )
        xg = np.zeros((NT, P, HO, max(TS)), dtype=bf)
        for ti in range(NT):
            tn = TS[ti]
            xg[ti, :, :, :tn] = xfull[:, :, tstarts[ti] : tstarts[ti] + tn]
        return xg

    def _unit_inputs(un, e, half, C, TS):
        f0, f1 = half * DFFH, (half + 1) * DFFH
        w1h = W1[e][:, f0:f1].astype(bf)          # [H, DFFH]
        m = {
            f"x{un}": _x_tiles(e, C, TS),
            f"w2{un}": np.ascontiguousarray(
                W2[e][f0:f1].astype(bf).reshape(FOH, P, H).transpose(1, 0, 2)
            ),
            f"b1{un}": np.ascontiguousarray(
                b1[e][f0:f1].reshape(FOH, P).T
            ).astype(np.float32),
            # b2 is added once per expert, by the front half only
            f"b2{un}": (
                np.ascontiguousarray(b2[e].reshape(HO, P).T).astype(np.float32)
                if half == 0
                else np.zeros((P, HO), dtype=np.float32)
            ),
        }
        wbu = np.zeros((P, C), dtype=np.float32)
        wbu[:, : len(tok_w[e])] = tok_w[e][None, :]
        m[f"wb{un}"] = wbu
        if un == "A":
            for fb in range(NFBLK):
                m[f"w1Ac{fb}"] = np.ascontiguousarray(
                    w1h[:, FBLK_STARTS[fb] : FBLK_STARTS[fb + 1]]
                    .reshape(HO, P, FBLKS[fb])
                    .transpose(1, 0, 2)
                )
        else:
            m["w1B"] = np.ascontiguousarray(
                w1h.reshape(HO, P, DFFH).transpose(1, 0, 2)
            )
        return m

    in_maps = []
    for pair in range(4):
        for half in range(2):
            m = {}
            m.update(_unit_inputs("A", heavy[pair], half, CA, TSA))
            m.update(_unit_inputs("B", light[pair], half, CB, TSB))
            in_maps.append(m)

    if not TRACE:
        # the agent image lacks antenv.axon_hooks; a stray BASS_TRACE in the
        # environment would crash the trace path, so disable it explicitly
        os.environ.setdefault("BASS_NEVER_TRACE", "1")

    # ---- sanity samples: 2 tokens per expert, recomputed on host ------------
    # Transient device-state episodes were observed to corrupt one core's
    # output (~10% of runs in one session window).  Checking the first and
    # last token routed to each expert validates every core's partials; on
    # mismatch the device run is retried with a freshly built program.
    def _gelu(v):
        return 0.5 * v * (1.0 + np.tanh(0.7978845608 * (v + 0.044715 * v**3)))

    chk_tok = sorted({int(t[j]) for t in tok_idx if len(t) for j in (0, -1)})
    chk_exp = np.zeros((len(chk_tok), H), dtype=np.float32)
    for n, t in enumerate(chk_tok):
        for e in (int(i1[t]), int(i2[t])):
            w = p[t, e] / denom[t]
            hm = _gelu(xf[t] @ W1[e] + b1[e])
            chk_exp[n] += np.float32(w) * (hm @ W2[e] + b2[e])
    chk_norm = np.maximum(np.linalg.norm(chk_exp, axis=1), 1e-3)

    out = None
    for attempt in range(3):
        # Rebuild the Bass program on every attempt: reusing an already-
        # lowered Bacc object across run_bass_kernel_spmd invocations
        # corrupts the second execution (NRT_EXEC_UNIT_UNRECOVERABLE).
        nc = _build(CA, TSA, CB, TSB, b2_zero=b2_zero)
        try:
            res = bass_utils.run_bass_kernel_spmd(
                nc, in_maps, core_ids=list(range(N_CORES)), trace=TRACE
            )
        except Exception:
            if attempt == 2:
                raise
            continue
        LAST_RESULTS = res

        # ---- combine (scatter-add of the weighted half-expert partials) -----
        out = np.zeros((T, H), dtype=np.float32)
        for pair in range(4):
            for un, e in (("A", heavy[pair]), ("B", light[pair])):
                cnt = len(tok_idx[e])
                if cnt:
                    part = (
                        res.results[2 * pair][f"out{un}"][:, :cnt].astype(np.float32)
                        + res.results[2 * pair + 1][f"out{un}"][:, :cnt].astype(
                            np.float32
                        )
                    )
                    out[tok_idx[e]] += part.T
        err = np.linalg.norm(out[chk_tok] - chk_exp, axis=1) / chk_norm
        if not len(err) or err.max() < 0.1:
            break
    return out.reshape(b, s, h)


# revision 20
# speedup vs baseline: 1.0055x; 1.0055x over previous
"""Expert-parallel MoE kernel for Trainium2 (8 NeuronCores).

Strategy (matches the module's intent):
  - Host computes the (tiny) gating: logits -> softmax -> top-2 -> renormalized
    combine weights. This is the router / all-to-all dispatch plumbing.
  - Expert e's weights (W1[e], b1[e], W2[e], b2[e]) live on core e.
  - Core e receives only its routed tokens (transposed, bf16) plus their
    combine weights, and computes  w * (gelu(x @ W1e + b1e) @ W2e + b2e)
    entirely on device (both matmuls in bf16 with fp32 PSUM accumulation).
  - Host scatter-adds the per-expert partial outputs back (the combine).

Layout: activations are kept feature-major on device (features on SBUF
partitions, tokens on the free dim) so both weight matrices are used in
their native layout as the stationary matmul operand and no transposes
are needed anywhere on device.

DMA schedule: everything the matmul stream consumes rides the sync-ring
HWDGE queue in consumption order (x slice 0, w1 chunks smallest-first,
x slice 1, w2, combine weights) so ring FIFO implements priority; w1
chunk sizes and the asymmetric token split are tuned so the ramping DMA
supply always stays ahead of the PE's consumption cadence.  Measured on
hardware: the 576-matmul stream runs gapless at ~113ns/matmul (the
N/2.4GHz+NX floor), so the kernel sits at the bf16 tensor-engine
roofline for its ~79 GFLOP/core; the remaining time is the framework
preamble (~7us), the DMA ramp to the first w1 chunk (~4us), and the
final store + epilogue (~4.5us).
"""

import os
import sys

sys.path.insert(0, "/opt/trn_rl_repo")

import numpy as np
import ml_dtypes

H = 768
E = 8
DFF = 3072
P = 128
HO = H // P      # 6 h-tiles
FO = DFF // P    # 24 f-tiles
N_CORES = 8
N_WARMUP_MM = 44  # dummy matmuls to open the HAM clock gate during DMA ramp
# w1 arrives in f-blocks; small leading blocks match the ramping DMA
# supply rate to the matmul consumption cadence (one 128-col j-group
# every ~0.69us) so the stream never stalls on a chunk arrival.
FBLKS = [128] * 8 + [256] * 4 + [512] * 2
FBLK_STARTS = [0]
for _c in FBLKS:
    FBLK_STARTS.append(FBLK_STARTS[-1] + _c)
assert FBLK_STARTS[-1] == DFF
NFBLK = len(FBLKS)
# j (128-col f-tile) -> (chunk index, col offset inside chunk)
J2FB = []
for _j in range(DFF // P):
    _c0 = _j * P
    for _fb in range(NFBLK):
        if FBLK_STARTS[_fb] <= _c0 < FBLK_STARTS[_fb + 1]:
            J2FB.append((_fb, _c0 - FBLK_STARTS[_fb]))
            break

LAST_RESULTS = None  # BassKernelResults of the most recent run (for test.py)
TRACE = False        # set True (e.g. by test.py) to profile the run


def _token_slices(C):
    """Split C tokens into PSUM-sized (<=512) slices.

    The split is asymmetric on purpose: slice 0 is ~65% so its matmul
    groups consume w1 chunks SLOWER than the ramping DMA supply
    delivers them (robustness against run-to-run DMA-ramp variance),
    and the final slice is small so the last output tile's store (on
    the kernel's critical tail) is cheap.
    """
    if C <= 512:
        return (C,)
    n_t = -(-C // 512)
    sizes = []
    left = C
    for k in range(n_t, 0, -1):
        if k == 1:
            s = left
        else:
            s = min(512, -(-int(left * 0.65) // 8) * 8)
        sizes.append(s)
        left -= s
    assert all(0 < s <= 512 for s in sizes) and sum(sizes) == C
    return tuple(sizes)


def _build(C, TS, act="gelu", b2_zero=False):
    import concourse.bass as bass
    import concourse.mybir as mybir
    import concourse.tile as tile
    from concourse import bacc

    f32 = mybir.dt.float32
    bf16 = mybir.dt.bfloat16
    GELU = (
        mybir.ActivationFunctionType.Gelu
        if act == "gelu"
        else mybir.ActivationFunctionType.Identity
    )
    IDENT = mybir.ActivationFunctionType.Identity

    nc = bacc.Bacc("TRN2", target_bir_lowering=False, debug=False)

    # Host passes everything pre-tiled so each DMA source is one contiguous
    # per-partition segment (max-size descriptors, minimal push cost).
    NT = len(TS)
    xT_d = nc.dram_tensor("xT", [NT, P, HO, max(TS)], bf16, kind="ExternalInput").ap()
    w1_d = [
        nc.dram_tensor(
            f"w1c{fb}", [P, HO, FBLKS[fb]], bf16, kind="ExternalInput"
        ).ap()
        for fb in range(NFBLK)
    ]
    w2_d = nc.dram_tensor("w2", [P, FO, H], bf16, kind="ExternalInput").ap()
    b1_d = nc.dram_tensor("b1", [P, FO], f32, kind="ExternalInput").ap()
    b2_d = nc.dram_tensor("b2", [P, HO], f32, kind="ExternalInput").ap()
    wb_d = nc.dram_tensor("wb", [P, C], f32, kind="ExternalInput").ap()
    # bf16 partial outputs halve the store traffic (host accumulates in
    # f32; the quantization adds ~0.2% rel err, well under the gate)
    out_dt = bf16 if b2_zero else f32
    out_d = nc.dram_tensor("outT", [H, C], out_dt, kind="ExternalOutput").ap()

    with tile.TileContext(nc) as tc:
        with (
            tc.tile_pool(name="const", bufs=1) as const,
            tc.tile_pool(name="hmidp", bufs=1) as hmidp,
            tc.tile_pool(name="psum", bufs=7, space="PSUM") as psum,
            tc.tile_pool(name="wupp", bufs=1, space="PSUM") as wupp,
            tc.tile_pool(name="outp", bufs=4) as outp,
        ):
            # ---- PE warm-up: dummy matmuls so the HAM clock-gate opens while
            # the weight DMAs are still in flight.  The memset runs on gpsimd
            # (it leaves the framework preamble ~1us earlier than vector).
            scr = const.tile([P, P], bf16, name="scr", tag="scr")
            nc.gpsimd.memset(scr, 0.0)
            psd = wupp.tile([P, P], f32, name="psd", tag="psd")
            for _ in range(N_WARMUP_MM):
                nc.tensor.matmul(psd, lhsT=scr, rhs=scr, start=True, stop=True)

            # ---- DMA schedule.  Everything the compute stream consumes goes
            # on the sync ring (queue 1) in consumption order: x slice 0, w1
            # chunks, x slice 1, w2, combine weights.  Ring FIFO = priority.
            # The ACT ring (queue 10) only gets the tiny b1: it has a 2-4.5us
            # startup latency and only ~130 GB/s, and the 8 DMAHW semaphore
            # lanes are shared across rings in scheduler-chosen order, so any
            # sizable transfer on the slow ring stalls later queue-1 issues
            # that land on the same lane (measured: +4us).  The combine
            # weights are pre-broadcast on the host to [P, C] — the DRE
            # replication broadcast ran at ~79 GB/s on the SWDGE queue and
            # stole SDMA time exactly while the first w1 chunks were in
            # flight.
            b1_sb = const.tile([P, FO], f32, name="b1_sb", tag="b1_sb")
            nc.scalar.dma_start(out=b1_sb, in_=b1_d)

            xT_sb = []
            for ti, tn in enumerate(TS):
                t = const.tile([P, HO, tn], bf16, name=f"xT{ti}", tag=f"xT{ti}")
                if ti == 0:
                    nc.sync.dma_start(out=t, in_=xT_d[ti, :, :, :tn])
                xT_sb.append(t)

            w1_sb = []
            for fb in range(NFBLK):
                cols = FBLKS[fb]
                t = const.tile([P, HO, cols], bf16, name=f"w1_{fb}", tag=f"w1_{fb}")
                nc.sync.dma_start(out=t, in_=w1_d[fb])
                w1_sb.append(t)

            for ti, tn in list(enumerate(TS))[1:]:
                nc.sync.dma_start(out=xT_sb[ti], in_=xT_d[ti, :, :, :tn])

            w2_sb = const.tile([P, FO, H], bf16, name="w2", tag="w2")
            nc.sync.dma_start(out=w2_sb, in_=w2_d)

            wb_sb = const.tile([P, C], f32, name="wb_sb", tag="wb_sb")
            nc.sync.dma_start(out=wb_sb, in_=wb_d)
            if not b2_zero:
                b2_sb = const.tile([P, HO], f32, name="b2_sb", tag="b2_sb")
                nc.scalar.dma_start(out=b2_sb, in_=b2_d)

            hmid_sb = [
                hmidp.tile([P, C], bf16, name=f"hmid{fo}", tag=f"hmid{fo}")
                for fo in range(FO)
            ]

            # ---- MLP layer 1: hmidT[f, t] = gelu(sum_h W1[h,f] xT[h,t] + b1[f]) --
            starts = np.cumsum([0] + list(TS))
            for ti, tn in enumerate(TS):
                t0 = int(starts[ti])
                for j in range(FO):
                    fb, joff = J2FB[j]
                    ps = psum.tile([P, 512], f32, name="ps1", tag="ps")
                    for ho in range(HO):
                        nc.tensor.matmul(
                            ps[:, :tn],
                            lhsT=w1_sb[fb][:, ho, joff : joff + P],
                            rhs=xT_sb[ti][:, ho, :tn],
                            start=(ho == 0),
                            stop=(ho == HO - 1),
                        )
                    nc.scalar.activation(
                        hmid_sb[j][:, t0 : t0 + tn],
                        ps[:, :tn],
                        GELU,
                        bias=b1_sb[:, j : j + 1],
                    )

            # ---- MLP layer 2 + combine scale ------------------------------------
            for ti, tn in enumerate(TS):
                t0 = int(starts[ti])
                for i in range(HO):
                    ps = psum.tile([P, 512], f32, name="ps2", tag="ps")
                    for fo in range(FO):
                        nc.tensor.matmul(
                            ps[:, :tn],
                            lhsT=w2_sb[:, fo, i * P : (i + 1) * P],
                            rhs=hmid_sb[fo][:, t0 : t0 + tn],
                            start=(fo == 0),
                            stop=(fo == FO - 1),
                        )
                    ot = outp.tile([P, 512], out_dt, name="ot", tag="ot")
                    if b2_zero:
                        # b2 == 0: evict with a single DVE multiply from PSUM
                        nc.vector.tensor_mul(
                            ot[:, :tn], ps[:, :tn], wb_sb[:, t0 : t0 + tn]
                        )
                    else:
                        nc.scalar.activation(
                            ot[:, :tn], ps[:, :tn], IDENT, bias=b2_sb[:, i : i + 1]
                        )
                        nc.vector.tensor_mul(
                            ot[:, :tn], ot[:, :tn], wb_sb[:, t0 : t0 + tn]
                        )
                    nc.sync.dma_start(
                        out=out_d[i * P : (i + 1) * P, t0 : t0 + tn], in_=ot[:, :tn]
                    )

    nc.compile()
    return nc


def kernel(x, Wg, bg, W1, b1, W2, b2, top_k):
    global LAST_RESULTS
    from concourse import bass_utils

    x = np.asarray(x, dtype=np.float32)
    Wg = np.asarray(Wg, dtype=np.float32)
    bg = np.asarray(bg, dtype=np.float32)
    W1 = np.asarray(W1, dtype=np.float32)
    b1 = np.asarray(b1, dtype=np.float32)
    W2 = np.asarray(W2, dtype=np.float32)
    b2 = np.asarray(b2, dtype=np.float32)
    k = int(np.asarray(top_k))
    assert k == 2, f"kernel specialized for top_k=2, got {k}"

    b, s, h = x.shape
    T = b * s
    xf = x.reshape(T, h)

    # ---- host router (the all-to-all dispatch) ------------------------------
    logits = xf @ Wg + bg
    m = logits.max(axis=-1, keepdims=True)
    p = np.exp(logits - m)
    p /= p.sum(axis=-1, keepdims=True)
    i1 = np.argmax(p, axis=-1)
    p_masked = p.copy()
    p_masked[np.arange(T), i1] = -np.inf
    i2 = np.argmax(p_masked, axis=-1)
    denom = p[np.arange(T), i1] + p[np.arange(T), i2]

    tok_idx, tok_w = [], []
    for e in range(E):
        sel = np.where((i1 == e) | (i2 == e))[0]
        tok_idx.append(sel.astype(np.int64))
        tok_w.append((p[sel, e] / denom[sel]).astype(np.float32))
    max_cnt = max(len(t) for t in tok_idx)
    C = max(-(-max_cnt // 16) * 16, 128)
    TS = _token_slices(C)

    b2_zero = not np.any(b2)

    # ---- per-core inputs ----------------------------------------------------
    bf = ml_dtypes.bfloat16
    in_maps = []
    for e in range(E):
        cnt = len(tok_idx[e])
        # xT t-slice-major [NT, P, HO, TSmax]: xg[t, p, o, c] = x[token, o*P+p]
        NT = len(TS)
        TSmax = max(TS)
        tstarts = np.concatenate([[0], np.cumsum(TS)]).astype(int)
        xfull = np.zeros((P, HO, C), dtype=bf)
        xfull[:, :, :cnt] = (
            np.ascontiguousarray(xf[tok_idx[e]].T).astype(bf)
            .reshape(HO, P, cnt)
            .transpose(1, 0, 2)
        )
        xg = np.zeros((NT, P, HO, TSmax), dtype=bf)
        for ti in range(NT):
            tn = TS[ti]
            xg[ti, :, :, :tn] = xfull[:, :, tstarts[ti] : tstarts[ti] + tn]
        wb = np.zeros((P, C), dtype=np.float32)
        wb[:, :cnt] = tok_w[e][None, :]
        # w1 chunks, each contiguous [P, HO, cols]
        w1bf = W1[e].astype(bf)
        w1cs = {
            f"w1c{fb}": np.ascontiguousarray(
                w1bf[:, FBLK_STARTS[fb] : FBLK_STARTS[fb + 1]]
                .reshape(HO, P, FBLKS[fb])
                .transpose(1, 0, 2)
            )
            for fb in range(NFBLK)
        }
        # w2 [P, FO, H]: w2[p, o, h] = W2[o*P+p, h]
        w2t = np.ascontiguousarray(
            W2[e].astype(bf).reshape(FO, P, H).transpose(1, 0, 2)
        )
        in_maps.append(
            {
                "xT": xg,
                **w1cs,
                "w2": w2t,
                "b1": np.ascontiguousarray(b1[e].reshape(FO, P).T),
                "b2": np.ascontiguousarray(b2[e].reshape(HO, P).T),
                "wb": wb,
            }
        )

    if not TRACE:
        # the agent image lacks antenv.axon_hooks; a stray BASS_TRACE in the
        # environment would crash the trace path, so disable it explicitly
        os.environ.setdefault("BASS_NEVER_TRACE", "1")

    # ---- sanity samples: 2 tokens per expert, recomputed on host ------------
    # Transient device-state episodes were observed to corrupt one core's
    # output (~10% of runs in one session window).  Checking the first and
    # last token routed to each expert validates every core's partial; on
    # mismatch the device run is retried with a freshly built program.
    def _gelu(v):
        return 0.5 * v * (1.0 + np.tanh(0.7978845608 * (v + 0.044715 * v**3)))

    chk_tok = sorted({int(t[k]) for t in tok_idx if len(t) for k in (0, -1)})
    chk_exp = np.zeros((len(chk_tok), H), dtype=np.float32)
    for n, t in enumerate(chk_tok):
        for e in (int(i1[t]), int(i2[t])):
            w = p[t, e] / denom[t]
            hm = _gelu(xf[t] @ W1[e] + b1[e])
            chk_exp[n] += np.float32(w) * (hm @ W2[e] + b2[e])
    chk_norm = np.maximum(np.linalg.norm(chk_exp, axis=1), 1e-3)

    out = None
    for attempt in range(3):
        # Rebuild the Bass program on every attempt: reusing an already-
        # lowered Bacc object across run_bass_kernel_spmd invocations
        # corrupts the second execution (NRT_EXEC_UNIT_UNRECOVERABLE).
        nc = _build(C, TS, b2_zero=b2_zero)
        try:
            res = bass_utils.run_bass_kernel_spmd(
                nc, in_maps, core_ids=list(range(N_CORES)), trace=TRACE
            )
        except Exception:
            if attempt == 2:
                raise
            continue
        LAST_RESULTS = res

        # ---- combine (scatter-add of the weighted expert outputs) -----------
        out = np.zeros((T, H), dtype=np.float32)
        for e in range(E):
            cnt = len(tok_idx[e])
            if cnt:
                out[tok_idx[e]] += (
                    res.results[e]["outT"][:, :cnt].T.astype(np.float32)
                )
        err = np.linalg.norm(out[chk_tok] - chk_exp, axis=1) / chk_norm
        if not len(err) or err.max() < 0.1:
            break
    return out.reshape(b, s, h)


# revision 21
# speedup vs baseline: 1.0071x; 1.0016x over previous
"""Expert-parallel MoE kernel for Trainium2 (8 NeuronCores).

Strategy (matches the module's intent):
  - Host computes the (tiny) gating: logits -> softmax -> top-2 -> renormalized
    combine weights. This is the router / all-to-all dispatch plumbing.
  - Expert e's weights (W1[e], b1[e], W2[e], b2[e]) live on core e.
  - Core e receives only its routed tokens (transposed, bf16) plus their
    combine weights, and computes  w * (gelu(x @ W1e + b1e) @ W2e + b2e)
    entirely on device (both matmuls in bf16 with fp32 PSUM accumulation).
  - Host scatter-adds the per-expert partial outputs back (the combine).

Layout: activations are kept feature-major on device (features on SBUF
partitions, tokens on the free dim) so both weight matrices are used in
their native layout as the stationary matmul operand and no transposes
are needed anywhere on device.

DMA schedule: everything the matmul stream consumes rides the sync-ring
HWDGE queue in consumption order (x slice 0, w1 chunks smallest-first,
x slice 1, w2, combine weights) so ring FIFO implements priority; w1
chunk sizes and the asymmetric token split are tuned so the ramping DMA
supply always stays ahead of the PE's consumption cadence.  Measured on
hardware: the 576-matmul stream runs gapless at ~113ns/matmul (the
N/2.4GHz+NX floor), so the kernel sits at the bf16 tensor-engine
roofline for its ~79 GFLOP/core; the remaining time is the framework
preamble (~7us), the DMA ramp to the first w1 chunk (~4us), and the
final store + epilogue (~4.5us).
"""

import os
import sys

sys.path.insert(0, "/opt/trn_rl_repo")

import numpy as np
import ml_dtypes

H = 768
E = 8
DFF = 3072
P = 128
HO = H // P      # 6 h-tiles
FO = DFF // P    # 24 f-tiles
N_CORES = 8
N_WARMUP_MM = 41  # dummy matmuls to open the HAM clock gate during DMA ramp
# w1 arrives in f-blocks; small leading blocks match the ramping DMA
# supply rate to the matmul consumption cadence (one 128-col j-group
# every ~0.69us) so the stream never stalls on a chunk arrival.
FBLKS = [128] * 8 + [256] * 4 + [512] * 2
FBLK_STARTS = [0]
for _c in FBLKS:
    FBLK_STARTS.append(FBLK_STARTS[-1] + _c)
assert FBLK_STARTS[-1] == DFF
NFBLK = len(FBLKS)
# j (128-col f-tile) -> (chunk index, col offset inside chunk)
J2FB = []
for _j in range(DFF // P):
    _c0 = _j * P
    for _fb in range(NFBLK):
        if FBLK_STARTS[_fb] <= _c0 < FBLK_STARTS[_fb + 1]:
            J2FB.append((_fb, _c0 - FBLK_STARTS[_fb]))
            break

LAST_RESULTS = None  # BassKernelResults of the most recent run (for test.py)
TRACE = False        # set True (e.g. by test.py) to profile the run


def _token_slices(C):
    """Split C tokens into PSUM-sized (<=512) slices.

    The split is asymmetric on purpose: slice 0 is ~65% so its matmul
    groups consume w1 chunks SLOWER than the ramping DMA supply
    delivers them (robustness against run-to-run DMA-ramp variance),
    and the final slice is small so the last output tile's store (on
    the kernel's critical tail) is cheap.
    """
    if C <= 512:
        return (C,)
    n_t = -(-C // 512)
    sizes = []
    left = C
    for k in range(n_t, 0, -1):
        if k == 1:
            s = left
        else:
            s = min(512, -(-int(left * 0.65) // 8) * 8)
        sizes.append(s)
        left -= s
    assert all(0 < s <= 512 for s in sizes) and sum(sizes) == C
    return tuple(sizes)


def _build(C, TS, act="gelu", b2_zero=False):
    import concourse.bass as bass
    import concourse.mybir as mybir
    import concourse.tile as tile
    from concourse import bacc

    f32 = mybir.dt.float32
    bf16 = mybir.dt.bfloat16
    GELU = (
        mybir.ActivationFunctionType.Gelu
        if act == "gelu"
        else mybir.ActivationFunctionType.Identity
    )
    IDENT = mybir.ActivationFunctionType.Identity

    nc = bacc.Bacc("TRN2", target_bir_lowering=False, debug=False)

    # Host passes everything pre-tiled so each DMA source is one contiguous
    # per-partition segment (max-size descriptors, minimal push cost).
    NT = len(TS)
    xT_d = nc.dram_tensor("xT", [NT, P, HO, max(TS)], bf16, kind="ExternalInput").ap()
    w1_d = [
        nc.dram_tensor(
            f"w1c{fb}", [P, HO, FBLKS[fb]], bf16, kind="ExternalInput"
        ).ap()
        for fb in range(NFBLK)
    ]
    w2_d = nc.dram_tensor("w2", [P, FO, H], bf16, kind="ExternalInput").ap()
    b1_d = nc.dram_tensor("b1", [P, FO], f32, kind="ExternalInput").ap()
    b2_d = nc.dram_tensor("b2", [P, HO], f32, kind="ExternalInput").ap()
    wb_d = nc.dram_tensor("wb", [P, C], f32, kind="ExternalInput").ap()
    # bf16 partial outputs halve the store traffic (host accumulates in
    # f32; the quantization adds ~0.2% rel err, well under the gate)
    out_dt = bf16 if b2_zero else f32
    out_d = nc.dram_tensor("outT", [H, C], out_dt, kind="ExternalOutput").ap()

    with tile.TileContext(nc) as tc:
        with (
            tc.tile_pool(name="const", bufs=1) as const,
            tc.tile_pool(name="hmidp", bufs=1) as hmidp,
            tc.tile_pool(name="psum", bufs=7, space="PSUM") as psum,
            tc.tile_pool(name="wupp", bufs=1, space="PSUM") as wupp,
            tc.tile_pool(name="outp", bufs=4) as outp,
        ):
            # ---- PE warm-up: dummy matmuls so the HAM clock-gate opens while
            # the weight DMAs are still in flight.  The memset runs on gpsimd
            # (it leaves the framework preamble ~1us earlier than vector).
            scr = const.tile([P, P], bf16, name="scr", tag="scr")
            nc.gpsimd.memset(scr, 0.0)
            psd = wupp.tile([P, P], f32, name="psd", tag="psd")
            for _ in range(N_WARMUP_MM):
                nc.tensor.matmul(psd, lhsT=scr, rhs=scr, start=True, stop=True)

            # ---- DMA schedule.  Everything the compute stream consumes goes
            # on the sync ring (queue 1) in consumption order: x slice 0, w1
            # chunks, x slice 1, w2, combine weights.  Ring FIFO = priority.
            # The ACT ring (queue 10) only gets the tiny b1: it has a 2-4.5us
            # startup latency and only ~130 GB/s, and the 8 DMAHW semaphore
            # lanes are shared across rings in scheduler-chosen order, so any
            # sizable transfer on the slow ring stalls later queue-1 issues
            # that land on the same lane (measured: +4us).  The combine
            # weights are pre-broadcast on the host to [P, C] — the DRE
            # replication broadcast ran at ~79 GB/s on the SWDGE queue and
            # stole SDMA time exactly while the first w1 chunks were in
            # flight.
            b1_sb = const.tile([P, FO], f32, name="b1_sb", tag="b1_sb")
            nc.scalar.dma_start(out=b1_sb, in_=b1_d)

            xT_sb = []
            for ti, tn in enumerate(TS):
                t = const.tile([P, HO, tn], bf16, name=f"xT{ti}", tag=f"xT{ti}")
                if ti == 0:
                    nc.sync.dma_start(out=t, in_=xT_d[ti, :, :, :tn])
                xT_sb.append(t)

            w1_sb = []
            for fb in range(NFBLK):
                cols = FBLKS[fb]
                t = const.tile([P, HO, cols], bf16, name=f"w1_{fb}", tag=f"w1_{fb}")
                nc.sync.dma_start(out=t, in_=w1_d[fb])
                w1_sb.append(t)

            for ti, tn in list(enumerate(TS))[1:]:
                nc.sync.dma_start(out=xT_sb[ti], in_=xT_d[ti, :, :, :tn])

            w2_sb = const.tile([P, FO, H], bf16, name="w2", tag="w2")
            nc.sync.dma_start(out=w2_sb, in_=w2_d)

            wb_sb = const.tile([P, C], f32, name="wb_sb", tag="wb_sb")
            nc.sync.dma_start(out=wb_sb, in_=wb_d)
            if not b2_zero:
                b2_sb = const.tile([P, HO], f32, name="b2_sb", tag="b2_sb")
                nc.scalar.dma_start(out=b2_sb, in_=b2_d)

            hmid_sb = [
                hmidp.tile([P, C], bf16, name=f"hmid{fo}", tag=f"hmid{fo}")
                for fo in range(FO)
            ]

            # ---- MLP layer 1: hmidT[f, t] = gelu(sum_h W1[h,f] xT[h,t] + b1[f]) --
            starts = np.cumsum([0] + list(TS))
            for ti, tn in enumerate(TS):
                t0 = int(starts[ti])
                for j in range(FO):
                    fb, joff = J2FB[j]
                    ps = psum.tile([P, 512], f32, name="ps1", tag="ps")
                    for ho in range(HO):
                        nc.tensor.matmul(
                            ps[:, :tn],
                            lhsT=w1_sb[fb][:, ho, joff : joff + P],
                            rhs=xT_sb[ti][:, ho, :tn],
                            start=(ho == 0),
                            stop=(ho == HO - 1),
                        )
                    nc.scalar.activation(
                        hmid_sb[j][:, t0 : t0 + tn],
                        ps[:, :tn],
                        GELU,
                        bias=b1_sb[:, j : j + 1],
                    )

            # ---- MLP layer 2 + combine scale ------------------------------------
            for ti, tn in enumerate(TS):
                t0 = int(starts[ti])
                for i in range(HO):
                    ps = psum.tile([P, 512], f32, name="ps2", tag="ps")
                    for fo in range(FO):
                        nc.tensor.matmul(
                            ps[:, :tn],
                            lhsT=w2_sb[:, fo, i * P : (i + 1) * P],
                            rhs=hmid_sb[fo][:, t0 : t0 + tn],
                            start=(fo == 0),
                            stop=(fo == FO - 1),
                        )
                    ot = outp.tile([P, 512], out_dt, name="ot", tag="ot")
                    if b2_zero:
                        # b2 == 0: evict with a single DVE multiply from PSUM
                        nc.vector.tensor_mul(
                            ot[:, :tn], ps[:, :tn], wb_sb[:, t0 : t0 + tn]
                        )
                    else:
                        nc.scalar.activation(
                            ot[:, :tn], ps[:, :tn], IDENT, bias=b2_sb[:, i : i + 1]
                        )
                        nc.vector.tensor_mul(
                            ot[:, :tn], ot[:, :tn], wb_sb[:, t0 : t0 + tn]
                        )
                    nc.sync.dma_start(
                        out=out_d[i * P : (i + 1) * P, t0 : t0 + tn], in_=ot[:, :tn]
                    )

    nc.compile()
    return nc


def kernel(x, Wg, bg, W1, b1, W2, b2, top_k):
    global LAST_RESULTS
    from concourse import bass_utils

    x = np.asarray(x, dtype=np.float32)
    Wg = np.asarray(Wg, dtype=np.float32)
    bg = np.asarray(bg, dtype=np.float32)
    W1 = np.asarray(W1, dtype=np.float32)
    b1 = np.asarray(b1, dtype=np.float32)
    W2 = np.asarray(W2, dtype=np.float32)
    b2 = np.asarray(b2, dtype=np.float32)
    k = int(np.asarray(top_k))
    assert k == 2, f"kernel specialized for top_k=2, got {k}"

    b, s, h = x.shape
    T = b * s
    xf = x.reshape(T, h)

    # ---- host router (the all-to-all dispatch) ------------------------------
    logits = xf @ Wg + bg
    m = logits.max(axis=-1, keepdims=True)
    p = np.exp(logits - m)
    p /= p.sum(axis=-1, keepdims=True)
    i1 = np.argmax(p, axis=-1)
    p_masked = p.copy()
    p_masked[np.arange(T), i1] = -np.inf
    i2 = np.argmax(p_masked, axis=-1)
    denom = p[np.arange(T), i1] + p[np.arange(T), i2]

    tok_idx, tok_w = [], []
    for e in range(E):
        sel = np.where((i1 == e) | (i2 == e))[0]
        tok_idx.append(sel.astype(np.int64))
        tok_w.append((p[sel, e] / denom[sel]).astype(np.float32))
    max_cnt = max(len(t) for t in tok_idx)
    C = max(-(-max_cnt // 16) * 16, 128)
    TS = _token_slices(C)

    b2_zero = not np.any(b2)

    # ---- per-core inputs ----------------------------------------------------
    bf = ml_dtypes.bfloat16
    in_maps = []
    for e in range(E):
        cnt = len(tok_idx[e])
        # xT t-slice-major [NT, P, HO, TSmax]: xg[t, p, o, c] = x[token, o*P+p]
        NT = len(TS)
        TSmax = max(TS)
        tstarts = np.concatenate([[0], np.cumsum(TS)]).astype(int)
        xfull = np.zeros((P, HO, C), dtype=bf)
        xfull[:, :, :cnt] = (
            np.ascontiguousarray(xf[tok_idx[e]].T).astype(bf)
            .reshape(HO, P, cnt)
            .transpose(1, 0, 2)
        )
        xg = np.zeros((NT, P, HO, TSmax), dtype=bf)
        for ti in range(NT):
            tn = TS[ti]
            xg[ti, :, :, :tn] = xfull[:, :, tstarts[ti] : tstarts[ti] + tn]
        wb = np.zeros((P, C), dtype=np.float32)
        wb[:, :cnt] = tok_w[e][None, :]
        # w1 chunks, each contiguous [P, HO, cols]
        w1bf = W1[e].astype(bf)
        w1cs = {
            f"w1c{fb}": np.ascontiguousarray(
                w1bf[:, FBLK_STARTS[fb] : FBLK_STARTS[fb + 1]]
                .reshape(HO, P, FBLKS[fb])
                .transpose(1, 0, 2)
            )
            for fb in range(NFBLK)
        }
        # w2 [P, FO, H]: w2[p, o, h] = W2[o*P+p, h]
        w2t = np.ascontiguousarray(
            W2[e].astype(bf).reshape(FO, P, H).transpose(1, 0, 2)
        )
        in_maps.append(
            {
                "xT": xg,
                **w1cs,
                "w2": w2t,
                "b1": np.ascontiguousarray(b1[e].reshape(FO, P).T),
                "b2": np.ascontiguousarray(b2[e].reshape(HO, P).T),
                "wb": wb,
            }
        )

    if not TRACE:
        # the agent image lacks antenv.axon_hooks; a stray BASS_TRACE in the
        # environment would crash the trace path, so disable it explicitly
        os.environ.setdefault("BASS_NEVER_TRACE", "1")

    # ---- sanity samples: 2 tokens per expert, recomputed on host ------------
    # Transient device-state episodes were observed to corrupt one core's
    # output (~10% of runs in one session window).  Checking the first and
    # last token routed to each expert validates every core's partial; on
    # mismatch the device run is retried with a freshly built program.
    def _gelu(v):
        return 0.5 * v * (1.0 + np.tanh(0.7978845608 * (v + 0.044715 * v**3)))

    chk_tok = sorted({int(t[k]) for t in tok_idx if len(t) for k in (0, -1)})
    chk_exp = np.zeros((len(chk_tok), H), dtype=np.float32)
    for n, t in enumerate(chk_tok):
        for e in (int(i1[t]), int(i2[t])):
            w = p[t, e] / denom[t]
            hm = _gelu(xf[t] @ W1[e] + b1[e])
            chk_exp[n] += np.float32(w) * (hm @ W2[e] + b2[e])
    chk_norm = np.maximum(np.linalg.norm(chk_exp, axis=1), 1e-3)

    out = None
    for attempt in range(3):
        # Rebuild the Bass program on every attempt: reusing an already-
        # lowered Bacc object across run_bass_kernel_spmd invocations
        # corrupts the second execution (NRT_EXEC_UNIT_UNRECOVERABLE).
        nc = _build(C, TS, b2_zero=b2_zero)
        try:
            res = bass_utils.run_bass_kernel_spmd(
                nc, in_maps, core_ids=list(range(N_CORES)), trace=TRACE
            )
        except Exception:
            if attempt == 2:
                raise
            continue
        LAST_RESULTS = res

        # ---- combine (scatter-add of the weighted expert outputs) -----------
        out = np.zeros((T, H), dtype=np.float32)
        for e in range(E):
            cnt = len(tok_idx[e])
            if cnt:
                out[tok_idx[e]] += (
                    res.results[e]["outT"][:, :cnt].T.astype(np.float32)
                )
        err = np.linalg.norm(out[chk_tok] - chk_exp, axis=1) / chk_norm
        if not len(err) or err.max() < 0.1:
            break
    return out.reshape(b, s, h)
